# revision 8
# baseline (speedup 1.0000x reference)
"""Bass/Trainium2 kernel for a binarized NN (BNN) forward pass, data-parallel
over 8 NeuronCores.

Reference semantics (fp32):
    h1 = x @ sign(W1).T;  b1 = sign(h1 - mean(h1, axis=0))        # g=1, b=0
    h2 = b1 @ sign(W2).T; b2 = noisy_sign(h2, u2)                  # BN+sign is
    h3 = b2 @ sign(W3).T; b3 = noisy_sign(h3, u3)                  # identity on +-1
    out = b3 @ sign(W4).T

Key implementation facts:
  * Layer 1 runs as a 2-pass fp16 matmul: x = xh + xl with xh = fp16(x),
    xl = fp16(x - xh).  sign(W1) is exact in fp16, so every product is exact
    and only fp32 PSUM accumulation rounds - error ~2^-23|x|, the same class
    as the reference's own fp32 matmul rounding.  The two passes are packed
    into one 1568-row contraction (padded to 13 chunks of 128) so the PE
    runs at 1 cycle/row (4x faster than fp32 mode's 4 cycles/row).
  * mean(h1) = sign(W1) @ mean(x) is computed on host in float64 and folded
    into the Sign activation bias (c1).
  * b in {+-1,0} and sign(W) in {+-1} make h2/h3/out exact small integers ->
    fp8 (e4m3) matmuls with DoubleRow perf mode are bit-exact.
  * batchnorm+sign on +-1 inputs is the identity, so layers 2/3 need no
    batch statistics and no cross-core communication.
  * The stochastic flip (u < 0.5*exp(-h^2/50)) & (|h| <= 50) with h an exact
    integer depends only on h and A(u) = smallest even a with p(a) <= u:
    flip <=> |h| < A.  One fused custom-DVE op computes the noisy sign in
    {+-1} directly:  with t = h - 1/4 and R2 = (A-1/2)^2 (or -1 when A = 0),
        noisy = clip(8 * t * (t*t - R2), -1, 1)
    which equals sign(t)*sign(|t| - (A-1/2)) = the exact noisy sign for every
    integer h (the 1/4 offset makes h=0 map to sign -1, as the reference
    does, and |t| is never 0 or equal to A-1/2).  R2 is exact where it
    matters even in bf16 (error << decision margins), so the u-derived
    tables ship as bf16, halving their HBM traffic.

Layout is feature-major: activations live as [features(partitions),
batch(free)].  Batch 16384 is sharded 2048/core; each core pipelines four
512-column slices through all four layers with layer chains interleaved at
chain granularity so the PE never waits on the DVE.
"""

from contextlib import ExitStack

import numpy as np

import concourse.bass as bass  # noqa: F401
import concourse.tile as tile
from concourse import bacc, mybir
from concourse.bass_utils import run_bass_kernel_spmd

F32 = mybir.dt.float32
F16 = mybir.dt.float16
BF16 = mybir.dt.bfloat16
FP8 = mybir.dt.float8e4
ACTF = mybir.ActivationFunctionType
DR = mybir.MatmulPerfMode.DoubleRow

N_CORES = 8
B = 16384                 # full batch
BC = B // N_CORES         # batch per core
D_IN = 784                # layer-1 input features
D_H = 1024                # hidden features
D_OUT = 10                # output features
D_PAD4 = 16               # L4 stationary dim padded for DoubleRow
KP = 13                   # packed fp16 k-chunks: 2*784 = 1568 -> 13*128 = 1664
K_PACK = 2 * D_IN         # rows of the packed (hi, lo) contraction
K_PAD = KP * 128
KH = D_H // 128           # 8 k-chunks for hidden layers
OC = D_H // 128           # 8 output-feature chunks
NT = BC // 256            # batch-column slices per core
NS = 256                  # slice width

# float32(0.5*exp(-(a*a)/50)) for a = 0,2,...,50 (bit-exact fallback table).
_PTABLE_BITS = [
    0x3F000000, 0x3EEC515A, 0x3EB9E4E3, 0x3E79375C, 0x3E0E5ACB, 0x3D8A9501,
    0x3CE5ED93, 0x3C2289CB, 0x3B43D285, 0x3A4909DD, 0x392FE09E, 0x38031DFC,
    0x36A696B8, 0x35345CD8, 0x33A6674D, 0x3202D2C5, 0x302F4A31, 0x2E4824C7,
    0x2C42BB52, 0x2A2173E9, 0x27E4229E, 0x258959AD, 0x230CEE5E, 0x207672F6,
    0x1DB79FE2, 0x1AE92B5E,
]


def _prob_table() -> np.ndarray:
    """p(a) for a = 0,2,...,50, bit-matching the reference's jnp.exp."""
    try:
        import jax.numpy as jnp

        a = np.arange(0, 51, 2, dtype=np.float32)
        p = np.asarray(0.5 * jnp.exp(-(jnp.asarray(a) * a) / (2.0 * 5.0**2)),
                       dtype=np.float32)
        if p.shape == (26,) and np.all(np.diff(p) < 0):
            return p
    except Exception:
        pass
    return np.array(_PTABLE_BITS, dtype=np.uint32).view(np.float32)


def _flip_thresholds(u: np.ndarray, ptable: np.ndarray) -> np.ndarray:
    """A(u): flip <=> |h| < A. A = 52 - 2 * #{a : p(a) <= u}."""
    tab = ptable[::-1].copy()  # ascending: p(50), p(48), ..., p(0)
    idx = np.searchsorted(tab, u, side="right")
    return (52 - 2 * idx).astype(np.float32)


def _r2_table(u: np.ndarray, ptable: np.ndarray) -> np.ndarray:
    """R2(u) for the fused noisy-sign op: (A-1/2)^2, or -1 when A = 0."""
    a = _flip_thresholds(u, ptable).astype(np.float64)
    r2 = np.where(a >= 2.0, (a - 0.5) ** 2, -1.0)
    return r2.astype(np.float32)


# ---------------------------------------------------------------------------
# Custom fused DVE op: noisy sign in one instruction.
#   out = clip(s1 * (in0-s0) * ((in0-s0)^2 - in1), -1, 1)
# With in0 = h (exact integer from PSUM), s0 = 0.25, s1 = 8, in1 = R2:
# out = sign(t)*sign(t^2 - R2) = the exact noisy sign in {+-1}.
# ---------------------------------------------------------------------------

_NOISY_OP_NAME = "NOISY_SIGN_PM1_ANT"


def _noisy_ref(in0, in1, c0, c1, c2):
    t = np.asarray(in0, np.float32) - np.float32(c0)
    w = (t * (t * t - np.asarray(in1, np.float32))) * np.float32(c1)
    return np.maximum(np.minimum(w, np.float32(1.0)), np.float32(-1.0))


def _register_noisy_op():
    from concourse import dve_ops
    from concourse.dve_spec import (C0, C1, One, Spec, Src0, Src1, Zero,
                                    lower, maxx, minn)
    from concourse.dve_uop import DveOpSpec

    for op in dve_ops.OPS:
        if op.name == _NOISY_OP_NAME:
            return op

    t = Src0 - C0
    w = (t * ((t * t) - Src1)) * C1
    body = maxx(minn(w, One), Zero - One)
    spec = Spec(body=body, reference=_noisy_ref)

    row = dve_ops._CUSTOM_DVE_ROW_BASE + len(dve_ops.OPS)
    assert row < 0x20, "custom-DVE opcode rows exhausted"
    shas = {}
    for ver in ("v3", "v4"):
        d = DveOpSpec(name=_NOISY_OP_NAME, opcode=row,
                      uops=lower(spec, ver=ver), rd1_en=True)
        shas[ver] = d.sha(ver)
    op = dve_ops.DveOp(_NOISY_OP_NAME, spec, subdim=False, uops_sha=shas)
    dve_ops.OPS.append(op)
    dve_ops.CUSTOM_DVE_SPECS[_NOISY_OP_NAME] = spec
    dve_ops._SUB_OPCODE_FOR_NAME[_NOISY_OP_NAME] = row
    return op


NOISY_OP = _register_noisy_op()


def build_nc(repeat: int = 1):
    """Build the per-core Bass program (same program on all 8 cores)."""
    nc = bacc.Bacc("TRN2", target_bir_lowering=False, debug=False,
                   num_devices=N_CORES)

    xt = [nc.dram_tensor(f"xt{n}", [128, KP, NS], F16,
                         kind="ExternalInput").ap() for n in range(NT)]
    w1 = [nc.dram_tensor(f"w1_{o}", [128, KP, 128], F16,
                         kind="ExternalInput").ap() for o in range(OC)]
    a2 = [nc.dram_tensor(f"a2_{n}", [128, OC, NS], BF16,
                         kind="ExternalInput").ap() for n in range(NT)]
    a3 = [nc.dram_tensor(f"a3_{n}", [128, OC, NS], BF16,
                         kind="ExternalInput").ap() for n in range(NT)]
    w2 = nc.dram_tensor("w2", [128, KH, D_H], FP8, kind="ExternalInput").ap()
    w3 = nc.dram_tensor("w3", [128, KH, D_H], FP8, kind="ExternalInput").ap()
    w4 = nc.dram_tensor("w4", [128, KH, D_PAD4], FP8,
                        kind="ExternalInput").ap()
    c1 = nc.dram_tensor("c1", [128, OC], F32, kind="ExternalInput").ap()
    out = nc.dram_tensor("out", [D_OUT, BC], F32, kind="ExternalOutput").ap()

    with tile.TileContext(nc) as tc:
        with ExitStack() as ctx:
            consts = ctx.enter_context(tc.tile_pool(name="consts", bufs=1))
            panels = ctx.enter_context(tc.tile_pool(name="panels", bufs=1))

            # Layer-1-critical loads first, all on the sync queue in priority
            # order (the DMA engines are a single shared resource): half of
            # w1's first block and half of the first xt slice let the first
            # matmul chain start ~3.5us in; the rest streams in behind it.
            c1_t = consts.tile([128, OC], F32, tag="c1")
            w1_t = consts.tile([128, OC * KP, 128], F16, tag="w1")
            xt_t = consts.tile([128, NT * KP, NS], F16, tag="xt")
            KPH = KP // 2
            nc.sync.dma_start(w1_t[:, 0:KPH, :], w1[0][:, 0:KPH, :])
            nc.sync.dma_start(c1_t[:], c1[:, :])
            nc.sync.dma_start(xt_t[:, 0:KPH, :], xt[0][:, 0:KPH, :])
            nc.sync.dma_start(w1_t[:, KPH:KP, :], w1[0][:, KPH:KP, :])
            nc.sync.dma_start(xt_t[:, KPH:KP, :], xt[0][:, KPH:KP, :])
            for o in range(1, OC):
                nc.sync.dma_start(w1_t[:, o * KP:(o + 1) * KP, :], w1[o])

            w2_t = consts.tile([128, KH, D_H], FP8, tag="w2")
            w3_t = consts.tile([128, KH, D_H], FP8, tag="w3")
            w4_t = consts.tile([128, KH, D_PAD4], FP8, tag="w4")
            nc.sync.dma_start(w4_t[:], w4[:, :, :])

            # +-1 activation panels, feature-major fp8.
            b1_t = panels.tile([128, KH, BC], FP8, tag="b1")
            b2_t = panels.tile([128, KH, BC], FP8, tag="b2")
            b3_t = panels.tile([128, KH, BC], FP8, tag="b3")

            for _rep in range(repeat):
                with ExitStack() as rep_ctx:
                    l1ps = rep_ctx.enter_context(
                        tc.tile_pool(name="l1ps", bufs=3, space="PSUM"))
                    l2ps = rep_ctx.enter_context(
                        tc.tile_pool(name="l2ps", bufs=2, space="PSUM"))
                    l3ps = rep_ctx.enter_context(
                        tc.tile_pool(name="l3ps", bufs=2, space="PSUM"))
                    l4ps = rep_ctx.enter_context(
                        tc.tile_pool(name="l4ps", bufs=1, space="PSUM"))
                    apool = rep_ctx.enter_context(
                        tc.tile_pool(name="apool", bufs=2))
                    opool = rep_ctx.enter_context(
                        tc.tile_pool(name="opool", bufs=2))

                    a2_t: dict[int, object] = {}
                    a3_t: dict[int, object] = {}
                    l4_pending = None  # (psum tile, slice index) across iters

                    # Software pipeline, skewed one slice per layer:
                    # iteration i runs L1(i) | L2(i-1) | L3(i-2), interleaved
                    # per output chunk so the PE always has a long L1 chain
                    # between short DR chains and never waits on the DVE.
                    # L4(i-2) rides one chain behind L3(i-2): its kp-th DR
                    # matmul needs only b3 chunk pair (2kp, 2kp+1), so it is
                    # emitted after chain 2kp+2; the last pair + PSUM copy +
                    # store run at the top of the next iteration.
                    for i in range(NT + 3):
                        # L4 leftovers from the previous iteration.
                        if l4_pending is not None:
                            ps4, n4 = l4_pending
                            s4 = slice(n4 * NS, (n4 + 1) * NS)
                            nc.tensor.matmul(
                                ps4[:], w4_t[:, KH - 2:KH, :],
                                b3_t[:, KH - 2:KH, s4],
                                start=False, stop=True, perf_mode=DR)
                            ot = opool.tile([D_OUT, NS], F32, tag="ot")
                            nc.scalar.activation(ot[:], ps4[:D_OUT, :],
                                                 ACTF.Copy)
                            nc.sync.dma_start(out[:, s4], ot[:])
                            l4_pending = None

                        # --- DMA prefetch for this iteration ---
                        if i + 1 < NT:
                            nc.sync.dma_start(
                                xt_t[:, (i + 1) * KP:(i + 2) * KP, :],
                                xt[i + 1])
                        if i == 0:
                            nc.sync.dma_start(w2_t[:], w2[:, :, :])
                        if i == 1:
                            nc.sync.dma_start(w3_t[:], w3[:, :, :])
                        if i < NT:
                            t_a2 = apool.tile([128, OC, NS], BF16, tag="a2")
                            nc.sync.dma_start(t_a2[:], a2[i])
                            a2_t[i] = t_a2
                        if 1 <= i <= NT:
                            t_a3 = apool.tile([128, OC, NS], BF16, tag="a3")
                            nc.sync.dma_start(t_a3[:], a3[i - 1])
                            a3_t[i - 1] = t_a3

                        n1, n2, n3 = i, i - 1, i - 2
                        s1 = slice(n1 * NS, (n1 + 1) * NS)
                        s2 = slice(n2 * NS, (n2 + 1) * NS)
                        s3 = slice(n3 * NS, (n3 + 1) * NS)

                        for o in range(OC):
                            if 0 <= n3 < NT and o >= 2 and o % 2 == 0:
                                kp = (o - 2) // 2
                                if kp == 0:
                                    ps4 = l4ps.tile([D_PAD4, NS], F32,
                                                    tag="mm4")
                                    l4_pending = (ps4, n3)
                                nc.tensor.matmul(
                                    ps4[:], w4_t[:, 2 * kp:2 * kp + 2, :],
                                    b3_t[:, 2 * kp:2 * kp + 2, s3],
                                    start=(kp == 0), stop=False,
                                    perf_mode=DR)
                            if n1 < NT:
                                ps = l1ps.tile([128, NS], F32, tag="mm1")
                                for k in range(KP):
                                    nc.tensor.matmul(
                                        ps[:],
                                        w1_t[:, o * KP + k, :],
                                        xt_t[:, n1 * KP + k, :],
                                        start=(k == 0),
                                        stop=(k == KP - 1),
                                    )
                                # b1 = sign(h1 - mu1); bias arrives negated.
                                nc.scalar.activation(
                                    b1_t[:, o, s1], ps[:], ACTF.Sign,
                                    bias=c1_t[:, o:o + 1])
                            if 0 <= n2 < NT:
                                ps = l2ps.tile([128, NS], F32, tag="mm2")
                                for kp in range(KH // 2):
                                    nc.tensor.matmul(
                                        ps[:],
                                        w2_t[:, 2 * kp:2 * kp + 2,
                                             o * 128:(o + 1) * 128],
                                        b1_t[:, 2 * kp:2 * kp + 2, s2],
                                        start=(kp == 0),
                                        stop=(kp == KH // 2 - 1),
                                        perf_mode=DR,
                                    )
                                nc.vector._custom_dve(
                                    NOISY_OP, out=b2_t[:, o, s2], in0=ps[:],
                                    in1=a2_t[n2][:, o, :], s0=0.25, s1=8.0)
                            if 0 <= n3 < NT:
                                ps = l3ps.tile([128, NS], F32, tag="mm3")
                                for kp in range(KH // 2):
                                    nc.tensor.matmul(
                                        ps[:],
                                        w3_t[:, 2 * kp:2 * kp + 2,
                                             o * 128:(o + 1) * 128],
                                        b2_t[:, 2 * kp:2 * kp + 2, s3],
                                        start=(kp == 0),
                                        stop=(kp == KH // 2 - 1),
                                        perf_mode=DR,
                                    )
                                nc.vector._custom_dve(
                                    NOISY_OP, out=b3_t[:, o, s3], in0=ps[:],
                                    in1=a3_t[n3][:, o, :], s0=0.25, s1=8.0)

    nc.compile()
    return nc


_NC_CACHE: dict[int, object] = {}


def _get_nc(repeat: int = 1):
    if repeat not in _NC_CACHE:
        _NC_CACHE[repeat] = build_nc(repeat)
    return _NC_CACHE[repeat]


def make_in_maps(x, u2, u3, W1, W2, W3, W4, **_unused):
    """Host preprocessing -> per-core input dicts."""
    fp8_np = mybir.dt.np(FP8)
    bf16_np = mybir.dt.np(BF16)

    x = np.ascontiguousarray(np.asarray(x, dtype=np.float32))
    W1b = np.sign(np.asarray(W1, dtype=np.float32))
    # mean(h1, axis=0) = sign(W1) @ mean(x, axis=0), in float64; negated so
    # the device computes Sign(h + bias) with bias = -mu1.
    mu1 = (W1b.astype(np.float64) @ x.mean(axis=0, dtype=np.float64)).astype(
        np.float32)
    c1 = np.ascontiguousarray((-mu1).reshape(OC, 128).T)  # [128, OC]

    # 2-pass fp16 split of x, packed as one zero-padded 1664-row contraction.
    xh = x.astype(np.float16)
    xl = (x - xh.astype(np.float32)).astype(np.float16)
    xt_all = np.zeros((K_PAD, B), dtype=np.float16)
    xt_all[:D_IN] = xh.T
    xt_all[D_IN:K_PACK] = xl.T

    w1p = np.zeros((K_PAD, D_H), dtype=np.float16)
    w1p[:D_IN] = W1b.T
    w1p[D_IN:K_PACK] = W1b.T
    # [o][p][k][m]: one contiguous DMA per 128-feature output block.
    w1_blocks = np.ascontiguousarray(
        w1p.reshape(KP, 128, OC, 128).transpose(2, 1, 0, 3))

    pt = _prob_table()
    r2_2 = _r2_table(np.asarray(u2), pt)   # [B, 1024]
    r2_3 = _r2_table(np.asarray(u3), pt)

    def _hidden_w(w):
        wt = np.sign(np.asarray(w, np.float32)).T.astype(fp8_np)  # [K, M]
        return np.ascontiguousarray(
            wt.reshape(KH, 128, wt.shape[1]).transpose(1, 0, 2))

    w2t = _hidden_w(W2)                    # [128, 8, 1024]
    w3t = _hidden_w(W3)
    w4t = _hidden_w(W4)                    # [128, 8, 10]
    w4p = np.zeros((128, KH, D_PAD4), dtype=fp8_np)
    w4p[:, :, :D_OUT] = w4t
    w4t = w4p

    in_maps = []
    for c in range(N_CORES):
        sl = slice(c * BC, (c + 1) * BC)
        m = {"w2": w2t, "w3": w3t, "w4": w4t, "c1": c1}
        xc = xt_all[:, sl]                 # [1664, 2048]
        v = xc.reshape(KP, 128, NT, NS).transpose(2, 1, 0, 3)
        for n in range(NT):
            m[f"xt{n}"] = np.ascontiguousarray(v[n])
        for o in range(OC):
            m[f"w1_{o}"] = w1_blocks[o]
        for nm, r2 in (("a2", r2_2), ("a3", r2_3)):
            rc = r2.T[:, sl].astype(bf16_np)         # [1024, 2048]
            rv = rc.reshape(OC, 128, NT, NS).transpose(2, 1, 0, 3)
            for n in range(NT):
                m[f"{nm}_{n}"] = np.ascontiguousarray(rv[n])
        in_maps.append(m)
    return in_maps


def kernel(x, u2, u3, W1, W2, W3, W4,
           g1=None, b1=None, g2=None, b2=None, g3=None, b3=None):
    for g in (g1, g2, g3):
        assert g is None or np.all(np.asarray(g) > 0), "kernel assumes g > 0"
    for b in (b1, b2, b3):
        assert b is None or np.all(np.asarray(b) == 0), "kernel assumes b == 0"

    nc = _get_nc(repeat=1)
    in_maps = make_in_maps(x, u2, u3, W1, W2, W3, W4)
    res = run_bass_kernel_spmd(nc, in_maps, core_ids=list(range(N_CORES)))

    out = np.empty((B, D_OUT), dtype=np.float32)
    for c in range(N_CORES):
        out[c * BC:(c + 1) * BC, :] = res.results[c]["out"].T
    return out


# revision 10
# speedup vs baseline: 1.0079x; 1.0079x over previous
"""Bass/Trainium2 kernel for a binarized NN (BNN) forward pass, data-parallel
over 8 NeuronCores.

Reference semantics (fp32):
    h1 = x @ sign(W1).T;  b1 = sign(h1 - mean(h1, axis=0))        # g=1, b=0
    h2 = b1 @ sign(W2).T; b2 = noisy_sign(h2, u2)                  # BN+sign is
    h3 = b2 @ sign(W3).T; b3 = noisy_sign(h3, u3)                  # identity on +-1
    out = b3 @ sign(W4).T

Key implementation facts:
  * Layer 1 runs as a 2-pass fp16 matmul: x = xh + xl with xh = fp16(x),
    xl = fp16(x - xh).  sign(W1) is exact in fp16, so every product is exact
    and only fp32 PSUM accumulation rounds - error ~2^-23|x|, the same class
    as the reference's own fp32 matmul rounding.  The two passes are packed
    into one 1568-row contraction (padded to 13 chunks of 128) so the PE
    runs at 1 cycle/row (4x faster than fp32 mode's 4 cycles/row).
  * mean(h1) = sign(W1) @ mean(x) is computed on host in float64 and folded
    into the Sign activation bias (c1).
  * b in {+-1,0} and sign(W) in {+-1} make h2/h3/out exact small integers ->
    fp8 (e4m3) matmuls with DoubleRow perf mode are bit-exact.
  * batchnorm+sign on +-1 inputs is the identity, so layers 2/3 need no
    batch statistics and no cross-core communication.
  * The stochastic flip (u < 0.5*exp(-h^2/50)) & (|h| <= 50) with h an exact
    integer depends only on h and A(u) = smallest even a with p(a) <= u:
    flip <=> |h| < A.  One fused custom-DVE op computes the noisy sign in
    {+-1} directly:  with t = h - 1/4 and R2 = (A-1/2)^2 (or -1 when A = 0),
        noisy = clip(8 * t * (t*t - R2), -1, 1)
    which equals sign(t)*sign(|t| - (A-1/2)) = the exact noisy sign for every
    integer h (the 1/4 offset makes h=0 map to sign -1, as the reference
    does, and |t| is never 0 or equal to A-1/2).  R2 is exact where it
    matters even in bf16 (error << decision margins), so the u-derived
    tables ship as bf16, halving their HBM traffic.

Layout is feature-major: activations live as [features(partitions),
batch(free)].  Batch 16384 is sharded 2048/core; each core pipelines four
512-column slices through all four layers with layer chains interleaved at
chain granularity so the PE never waits on the DVE.
"""

from contextlib import ExitStack

import numpy as np

import concourse.bass as bass  # noqa: F401
import concourse.tile as tile
from concourse import bacc, mybir
from concourse.bass_utils import run_bass_kernel_spmd

F32 = mybir.dt.float32
F16 = mybir.dt.float16
BF16 = mybir.dt.bfloat16
FP8 = mybir.dt.float8e4
ACTF = mybir.ActivationFunctionType
DR = mybir.MatmulPerfMode.DoubleRow

N_CORES = 8
B = 16384                 # full batch
BC = B // N_CORES         # batch per core
D_IN = 784                # layer-1 input features
D_H = 1024                # hidden features
D_OUT = 10                # output features
D_PAD4 = 16               # L4 stationary dim padded for DoubleRow
KP = 13                   # packed fp16 k-chunks: 2*784 = 1568 -> 13*128 = 1664
K_PACK = 2 * D_IN         # rows of the packed (hi, lo) contraction
K_PAD = KP * 128
KH = D_H // 128           # 8 k-chunks for hidden layers
OC = D_H // 128           # 8 output-feature chunks
NT = BC // 256            # batch-column slices per core
NS = 256                  # slice width

# float32(0.5*exp(-(a*a)/50)) for a = 0,2,...,50 (bit-exact fallback table).
_PTABLE_BITS = [
    0x3F000000, 0x3EEC515A, 0x3EB9E4E3, 0x3E79375C, 0x3E0E5ACB, 0x3D8A9501,
    0x3CE5ED93, 0x3C2289CB, 0x3B43D285, 0x3A4909DD, 0x392FE09E, 0x38031DFC,
    0x36A696B8, 0x35345CD8, 0x33A6674D, 0x3202D2C5, 0x302F4A31, 0x2E4824C7,
    0x2C42BB52, 0x2A2173E9, 0x27E4229E, 0x258959AD, 0x230CEE5E, 0x207672F6,
    0x1DB79FE2, 0x1AE92B5E,
]


def _prob_table() -> np.ndarray:
    """p(a) for a = 0,2,...,50, bit-matching the reference's jnp.exp."""
    try:
        import jax.numpy as jnp

        a = np.arange(0, 51, 2, dtype=np.float32)
        p = np.asarray(0.5 * jnp.exp(-(jnp.asarray(a) * a) / (2.0 * 5.0**2)),
                       dtype=np.float32)
        if p.shape == (26,) and np.all(np.diff(p) < 0):
            return p
    except Exception:
        pass
    return np.array(_PTABLE_BITS, dtype=np.uint32).view(np.float32)


def _flip_thresholds(u: np.ndarray, ptable: np.ndarray) -> np.ndarray:
    """A(u): flip <=> |h| < A. A = 52 - 2 * #{a : p(a) <= u}."""
    tab = ptable[::-1].copy()  # ascending: p(50), p(48), ..., p(0)
    idx = np.searchsorted(tab, u, side="right")
    return (52 - 2 * idx).astype(np.float32)


def _r2_table(u: np.ndarray, ptable: np.ndarray) -> np.ndarray:
    """R2(u) for the fused noisy-sign op: (A-1/2)^2, or -1 when A = 0."""
    a = _flip_thresholds(u, ptable).astype(np.float64)
    r2 = np.where(a >= 2.0, (a - 0.5) ** 2, -1.0)
    return r2.astype(np.float32)


# ---------------------------------------------------------------------------
# Custom fused DVE op: noisy sign in one instruction.
#   out = clip(s1 * (in0-s0) * ((in0-s0)^2 - in1), -1, 1)
# With in0 = h (exact integer from PSUM), s0 = 0.25, s1 = 8, in1 = R2:
# out = sign(t)*sign(t^2 - R2) = the exact noisy sign in {+-1}.
# ---------------------------------------------------------------------------

_NOISY_OP_NAME = "NOISY_SIGN_PM1_ANT"


def _noisy_ref(in0, in1, c0, c1, c2):
    t = np.asarray(in0, np.float32) - np.float32(c0)
    w = (t * (t * t - np.asarray(in1, np.float32))) * np.float32(c1)
    return np.maximum(np.minimum(w, np.float32(1.0)), np.float32(-1.0))


def _register_noisy_op():
    from concourse import dve_ops
    from concourse.dve_spec import (C0, C1, One, Spec, Src0, Src1, Zero,
                                    lower, maxx, minn)
    from concourse.dve_uop import DveOpSpec

    for op in dve_ops.OPS:
        if op.name == _NOISY_OP_NAME:
            return op

    t = Src0 - C0
    w = (t * ((t * t) - Src1)) * C1
    body = maxx(minn(w, One), Zero - One)
    spec = Spec(body=body, reference=_noisy_ref)

    row = dve_ops._CUSTOM_DVE_ROW_BASE + len(dve_ops.OPS)
    assert row < 0x20, "custom-DVE opcode rows exhausted"
    shas = {}
    for ver in ("v3", "v4"):
        d = DveOpSpec(name=_NOISY_OP_NAME, opcode=row,
                      uops=lower(spec, ver=ver), rd1_en=True)
        shas[ver] = d.sha(ver)
    op = dve_ops.DveOp(_NOISY_OP_NAME, spec, subdim=False, uops_sha=shas)
    dve_ops.OPS.append(op)
    dve_ops.CUSTOM_DVE_SPECS[_NOISY_OP_NAME] = spec
    dve_ops._SUB_OPCODE_FOR_NAME[_NOISY_OP_NAME] = row
    return op


NOISY_OP = _register_noisy_op()


def build_nc(repeat: int = 1):
    """Build the per-core Bass program (same program on all 8 cores)."""
    nc = bacc.Bacc("TRN2", target_bir_lowering=False, debug=False,
                   num_devices=N_CORES)

    xt = [nc.dram_tensor(f"xt{n}", [128, KP, NS], F16,
                         kind="ExternalInput").ap() for n in range(NT)]
    w1 = [nc.dram_tensor(f"w1_{o}", [128, KP, 128], F16,
                         kind="ExternalInput").ap() for o in range(OC)]
    a2 = [nc.dram_tensor(f"a2_{n}", [128, OC, NS], BF16,
                         kind="ExternalInput").ap() for n in range(NT)]
    a3 = [nc.dram_tensor(f"a3_{n}", [128, OC, NS], BF16,
                         kind="ExternalInput").ap() for n in range(NT)]
    w2 = nc.dram_tensor("w2", [128, KH, D_H], FP8, kind="ExternalInput").ap()
    w3 = nc.dram_tensor("w3", [128, KH, D_H], FP8, kind="ExternalInput").ap()
    w4 = nc.dram_tensor("w4", [128, KH, D_PAD4], FP8,
                        kind="ExternalInput").ap()
    c1 = nc.dram_tensor("c1", [128, OC], F32, kind="ExternalInput").ap()
    out = nc.dram_tensor("out", [D_OUT, BC], F32, kind="ExternalOutput").ap()

    with tile.TileContext(nc) as tc:
        with ExitStack() as ctx:
            consts = ctx.enter_context(tc.tile_pool(name="consts", bufs=1))
            panels = ctx.enter_context(tc.tile_pool(name="panels", bufs=1))

            # Layer-1-critical loads first, all on the sync queue in priority
            # order (the DMA engines are a single shared resource): half of
            # w1's first block and half of the first xt slice let the first
            # matmul chain start ~3.5us in; the rest streams in behind it.
            c1_t = consts.tile([128, OC], F32, tag="c1")
            w1_t = consts.tile([128, OC * KP, 128], F16, tag="w1")
            xt_t = consts.tile([128, NT * KP, NS], F16, tag="xt")
            KPH = KP // 2
            nc.sync.dma_start(w1_t[:, 0:KPH, :], w1[0][:, 0:KPH, :])
            nc.sync.dma_start(c1_t[:], c1[:, :])
            nc.sync.dma_start(xt_t[:, 0:KPH, :], xt[0][:, 0:KPH, :])
            nc.sync.dma_start(w1_t[:, KPH:KP, :], w1[0][:, KPH:KP, :])
            nc.sync.dma_start(xt_t[:, KPH:KP, :], xt[0][:, KPH:KP, :])
            for o in range(1, OC):
                nc.sync.dma_start(w1_t[:, o * KP:(o + 1) * KP, :], w1[o])

            w2_t = consts.tile([128, KH, D_H], FP8, tag="w2")
            w3_t = consts.tile([128, KH, D_H], FP8, tag="w3")
            w4_t = consts.tile([128, KH, D_PAD4], FP8, tag="w4")
            nc.sync.dma_start(w4_t[:], w4[:, :, :])

            # +-1 activation panels, feature-major fp8.
            b1_t = panels.tile([128, KH, BC], FP8, tag="b1")
            b2_t = panels.tile([128, KH, BC], FP8, tag="b2")
            b3_t = panels.tile([128, KH, BC], FP8, tag="b3")

            for _rep in range(repeat):
                with ExitStack() as rep_ctx:
                    l1ps = rep_ctx.enter_context(
                        tc.tile_pool(name="l1ps", bufs=3, space="PSUM"))
                    l2ps = rep_ctx.enter_context(
                        tc.tile_pool(name="l2ps", bufs=2, space="PSUM"))
                    l3ps = rep_ctx.enter_context(
                        tc.tile_pool(name="l3ps", bufs=2, space="PSUM"))
                    l4ps = rep_ctx.enter_context(
                        tc.tile_pool(name="l4ps", bufs=1, space="PSUM"))
                    apool = rep_ctx.enter_context(
                        tc.tile_pool(name="apool", bufs=2))
                    opool = rep_ctx.enter_context(
                        tc.tile_pool(name="opool", bufs=2))

                    a2_t: dict[int, object] = {}
                    a3_t: dict[int, object] = {}
                    l4_pending = None  # (psum tile, slice index) across iters

                    # Software pipeline, skewed one slice per layer:
                    # iteration i runs L1(i) | L2(i-1) | L3(i-2), interleaved
                    # per output chunk so the PE always has a long L1 chain
                    # between short DR chains and never waits on the DVE.
                    # L4(i-2) rides one chain behind L3(i-2): its kp-th DR
                    # matmul needs only b3 chunk pair (2kp, 2kp+1), so it is
                    # emitted after chain 2kp+2; the last pair + PSUM copy +
                    # store run at the top of the next iteration.
                    for i in range(NT + 3):
                        # L4 leftovers from the previous iteration.
                        if l4_pending is not None:
                            ps4, n4 = l4_pending
                            s4 = slice(n4 * NS, (n4 + 1) * NS)
                            nc.tensor.matmul(
                                ps4[:], w4_t[:, KH - 2:KH, :],
                                b3_t[:, KH - 2:KH, s4],
                                start=False, stop=True, perf_mode=DR)
                            ot = opool.tile([D_OUT, NS], F32, tag="ot")
                            nc.scalar.activation(ot[:], ps4[:D_OUT, :],
                                                 ACTF.Copy)
                            nc.sync.dma_start(out[:, s4], ot[:])
                            l4_pending = None

                        # --- DMA prefetch for this iteration ---
                        if i + 1 < NT:
                            nc.sync.dma_start(
                                xt_t[:, (i + 1) * KP:(i + 2) * KP, :],
                                xt[i + 1])
                        if i == 0:
                            nc.sync.dma_start(w2_t[:], w2[:, :, :])
                        if i == 1:
                            nc.sync.dma_start(w3_t[:], w3[:, :, :])
                        if i < NT:
                            t_a2 = apool.tile([128, OC, NS], BF16, tag="a2")
                            nc.sync.dma_start(t_a2[:], a2[i])
                            a2_t[i] = t_a2
                        if 1 <= i <= NT:
                            t_a3 = apool.tile([128, OC, NS], BF16, tag="a3")
                            nc.sync.dma_start(t_a3[:], a3[i - 1])
                            a3_t[i - 1] = t_a3

                        n1, n2, n3 = i, i - 1, i - 2
                        s1 = slice(n1 * NS, (n1 + 1) * NS)
                        s2 = slice(n2 * NS, (n2 + 1) * NS)
                        s3 = slice(n3 * NS, (n3 + 1) * NS)

                        for o in range(OC):
                            if n1 < NT:
                                ps = l1ps.tile([128, NS], F32, tag="mm1")
                                for k in range(KP):
                                    nc.tensor.matmul(
                                        ps[:],
                                        w1_t[:, o * KP + k, :],
                                        xt_t[:, n1 * KP + k, :],
                                        start=(k == 0),
                                        stop=(k == KP - 1),
                                    )
                                # b1 = sign(h1 - mu1); bias arrives negated.
                                nc.scalar.activation(
                                    b1_t[:, o, s1], ps[:], ACTF.Sign,
                                    bias=c1_t[:, o:o + 1])
                            if 0 <= n3 < NT and o >= 2 and o % 2 == 0:
                                # L4(n3) rides one chain behind L3(n3); the
                                # L1 chain above hides the DVE latency.
                                kp = (o - 2) // 2
                                if kp == 0:
                                    ps4 = l4ps.tile([D_PAD4, NS], F32,
                                                    tag="mm4")
                                    l4_pending = (ps4, n3)
                                nc.tensor.matmul(
                                    ps4[:], w4_t[:, 2 * kp:2 * kp + 2, :],
                                    b3_t[:, 2 * kp:2 * kp + 2, s3],
                                    start=(kp == 0), stop=False,
                                    perf_mode=DR)
                            if 0 <= n2 < NT:
                                ps = l2ps.tile([128, NS], F32, tag="mm2")
                                for kp in range(KH // 2):
                                    nc.tensor.matmul(
                                        ps[:],
                                        w2_t[:, 2 * kp:2 * kp + 2,
                                             o * 128:(o + 1) * 128],
                                        b1_t[:, 2 * kp:2 * kp + 2, s2],
                                        start=(kp == 0),
                                        stop=(kp == KH // 2 - 1),
                                        perf_mode=DR,
                                    )
                                nc.vector._custom_dve(
                                    NOISY_OP, out=b2_t[:, o, s2], in0=ps[:],
                                    in1=a2_t[n2][:, o, :], s0=0.25, s1=8.0)
                            if 0 <= n3 < NT:
                                ps = l3ps.tile([128, NS], F32, tag="mm3")
                                for kp in range(KH // 2):
                                    nc.tensor.matmul(
                                        ps[:],
                                        w3_t[:, 2 * kp:2 * kp + 2,
                                             o * 128:(o + 1) * 128],
                                        b2_t[:, 2 * kp:2 * kp + 2, s3],
                                        start=(kp == 0),
                                        stop=(kp == KH // 2 - 1),
                                        perf_mode=DR,
                                    )
                                nc.vector._custom_dve(
                                    NOISY_OP, out=b3_t[:, o, s3], in0=ps[:],
                                    in1=a3_t[n3][:, o, :], s0=0.25, s1=8.0)

    nc.compile()
    return nc


_NC_CACHE: dict[int, object] = {}


def _get_nc(repeat: int = 1):
    if repeat not in _NC_CACHE:
        _NC_CACHE[repeat] = build_nc(repeat)
    return _NC_CACHE[repeat]


def make_in_maps(x, u2, u3, W1, W2, W3, W4, **_unused):
    """Host preprocessing -> per-core input dicts."""
    fp8_np = mybir.dt.np(FP8)
    bf16_np = mybir.dt.np(BF16)

    x = np.ascontiguousarray(np.asarray(x, dtype=np.float32))
    W1b = np.sign(np.asarray(W1, dtype=np.float32))
    # mean(h1, axis=0) = sign(W1) @ mean(x, axis=0), in float64; negated so
    # the device computes Sign(h + bias) with bias = -mu1.
    mu1 = (W1b.astype(np.float64) @ x.mean(axis=0, dtype=np.float64)).astype(
        np.float32)
    c1 = np.ascontiguousarray((-mu1).reshape(OC, 128).T)  # [128, OC]

    # 2-pass fp16 split of x, packed as one zero-padded 1664-row contraction.
    xh = x.astype(np.float16)
    xl = (x - xh.astype(np.float32)).astype(np.float16)
    xt_all = np.zeros((K_PAD, B), dtype=np.float16)
    xt_all[:D_IN] = xh.T
    xt_all[D_IN:K_PACK] = xl.T

    w1p = np.zeros((K_PAD, D_H), dtype=np.float16)
    w1p[:D_IN] = W1b.T
    w1p[D_IN:K_PACK] = W1b.T
    # [o][p][k][m]: one contiguous DMA per 128-feature output block.
    w1_blocks = np.ascontiguousarray(
        w1p.reshape(KP, 128, OC, 128).transpose(2, 1, 0, 3))

    pt = _prob_table()
    r2_2 = _r2_table(np.asarray(u2), pt)   # [B, 1024]
    r2_3 = _r2_table(np.asarray(u3), pt)

    def _hidden_w(w):
        wt = np.sign(np.asarray(w, np.float32)).T.astype(fp8_np)  # [K, M]
        return np.ascontiguousarray(
            wt.reshape(KH, 128, wt.shape[1]).transpose(1, 0, 2))

    w2t = _hidden_w(W2)                    # [128, 8, 1024]
    w3t = _hidden_w(W3)
    w4t = _hidden_w(W4)                    # [128, 8, 10]
    w4p = np.zeros((128, KH, D_PAD4), dtype=fp8_np)
    w4p[:, :, :D_OUT] = w4t
    w4t = w4p

    in_maps = []
    for c in range(N_CORES):
        sl = slice(c * BC, (c + 1) * BC)
        m = {"w2": w2t, "w3": w3t, "w4": w4t, "c1": c1}
        xc = xt_all[:, sl]                 # [1664, 2048]
        v = xc.reshape(KP, 128, NT, NS).transpose(2, 1, 0, 3)
        for n in range(NT):
            m[f"xt{n}"] = np.ascontiguousarray(v[n])
        for o in range(OC):
            m[f"w1_{o}"] = w1_blocks[o]
        for nm, r2 in (("a2", r2_2), ("a3", r2_3)):
            rc = r2.T[:, sl].astype(bf16_np)         # [1024, 2048]
            rv = rc.reshape(OC, 128, NT, NS).transpose(2, 1, 0, 3)
            for n in range(NT):
                m[f"{nm}_{n}"] = np.ascontiguousarray(rv[n])
        in_maps.append(m)
    return in_maps


def kernel(x, u2, u3, W1, W2, W3, W4,
           g1=None, b1=None, g2=None, b2=None, g3=None, b3=None):
    for g in (g1, g2, g3):
        assert g is None or np.all(np.asarray(g) > 0), "kernel assumes g > 0"
    for b in (b1, b2, b3):
        assert b is None or np.all(np.asarray(b) == 0), "kernel assumes b == 0"

    nc = _get_nc(repeat=1)
    in_maps = make_in_maps(x, u2, u3, W1, W2, W3, W4)
    res = run_bass_kernel_spmd(nc, in_maps, core_ids=list(range(N_CORES)))

    out = np.empty((B, D_OUT), dtype=np.float32)
    for c in range(N_CORES):
        out[c * BC:(c + 1) * BC, :] = res.results[c]["out"].T
    return out
